# revision 20
# baseline (speedup 1.0000x reference)
# Trainium2 Bass kernel for nn_NetSparse1 (topk_masking).
#
# Computes: log_softmax( relu(x @ (w1*m1).T) @ (w2*m2).T ) where m1/m2 are
# top-50%-|score| masks (GetSubnetEP semantics, stable-sort tie handling).
#
# Strategy (data-parallel over 8 NeuronCores, batch dim sharded):
#   host: compute the exact GetSubnetEP masks (k-th order statistic +
#         stable-sort tie handling) and apply them to the weights, then
#         quantize: layer-1 weights and x to fp8e4 (e4m3) in the PE's
#         DoubleRow pair layout (2x matmul throughput), 16-row K-tail and
#         layer-2 weights in bf16. Masking is a pure function of the
#         (replicated) weights/scores, so no per-batch work happens here.
#   device (per core, 2048 batch rows):
#     main: hc-outer / bb-inner: per 128-hidden chunk and 512-batch block,
#           psum[128h,512b] += w1q_pair.T @ xq_pair via 3 fp8 DoubleRow
#           matmuls (K=256 each, 768 of 784), then the 16-row bf16
#           K-remainder of all 4 batch blocks concurrently in PE row-groups
#           0/32/64/96. relu -> bf16 ht split across ACT/DVE/Pool so no
#           single engine gates the PE. logitsT[10,512] += w2m_chunk.T @ ht,
#           deferred one full chunk so the PE never stalls on the relu.
#           A short bf16 warmup matmul chain keeps the HAM clock-gate at
#           K=8/8 from the start.
#     epilog: batched log_softmax over 16 [128,10] tiles (PE transpose,
#           max-shift, Exp/Ln grouped to avoid ACT table swaps), one DMA.
# No collectives needed; host concatenates the 8 per-core outputs.

import numpy as np
import ml_dtypes

import concourse.bass as bass
import concourse.tile as tile
from concourse import bacc, mybir
from concourse.bass_utils import run_bass_kernel_spmd
from concourse.masks import make_identity

N_CORES = 8
B = 16384
BC = B // N_CORES      # 2048 batch rows per core
IN_DIM = 784
HIDDEN = 8192
OUT_DIM = 10
OUT_PAD = 16          # layer-2 N padded so dual-fp8 ldweights strides stay even
SPARSITY = 0.5

P = 128
KP = 3                 # fp8 DoubleRow K-pairs (3 x 256 = 768 of 784)
K_LAST = IN_DIM - KP * 2 * P  # 16-row bf16 remainder
HC = HIDDEN // P       # 64 hidden chunks
HCP = HC // 2          # 32 hidden chunk pairs (fp8 DoubleRow layer 2)
BB = 512               # batch block (PSUM free dim)
NBB = BC // BB         # 4
W1_PIECES = 8          # w1q DMA pieces along hidden
HC_PER_PIECE = HC // W1_PIECES

F32 = mybir.dt.float32
BF16 = mybir.dt.bfloat16
FP8 = mybir.dt.float8e4

_BF16 = ml_dtypes.bfloat16
_FP8 = ml_dtypes.float8_e4m3

DR = mybir.MatmulPerfMode.DoubleRow


def _build_nc():
    nc = bacc.Bacc("TRN2")

    xq = nc.dram_tensor("xq", (P, KP, 2, BC), FP8, kind="ExternalInput")
    xt = nc.dram_tensor("xt", (P, BC), FP8, kind="ExternalInput")
    w1q = nc.dram_tensor("w1q", (P, KP, 2, HIDDEN), FP8, kind="ExternalInput")
    w1t = nc.dram_tensor("w1t", (P, HIDDEN), FP8, kind="ExternalInput")
    w2q = nc.dram_tensor("w2q", (P, HCP, 2, OUT_PAD), FP8, kind="ExternalInput")
    out = nc.dram_tensor("out", (BC, OUT_DIM), F32, kind="ExternalOutput")

    with tile.TileContext(nc) as tc:
        with (
            tc.tile_pool(name="singles", bufs=1) as singles,
            tc.tile_pool(name="wres", bufs=1) as wres,
            tc.tile_pool(name="hpool", bufs=8) as hpool,
            tc.tile_pool(name="opool", bufs=4) as opool,
            tc.tile_pool(name="tailp", bufs=1) as tailp,
            tc.tile_pool(name="psh", bufs=4, space=bass.MemorySpace.PSUM) as psh,
            tc.tile_pool(name="psl", bufs=1, space=bass.MemorySpace.PSUM) as psl,
        ):
            # zero bias for activations
            zb = singles.tile([P, 1], F32, tag="zb")
            nc.vector.memset(zb, 0.0)

            # identity for PE transpose
            ident = singles.tile([P, P], F32, tag="ident")
            make_identity(nc, ident[:])

            # PE warmup: dependency-free bf16 matmul chain so the HAM
            # clock-gate is at K=8/8 when the first real matmul's inputs land
            wz = singles.tile([P, 2, BB], FP8, tag="wz")
            nc.vector.memset(wz, 0.0)
            warm = psh.tile([P, BB], F32, tag="ph")
            NWARM = 44
            for i in range(NWARM):
                nc.tensor.matmul(warm, wz[:, :, :P], wz, start=(i == 0),
                                 stop=(i == NWARM - 1), perf_mode=DR)

            # x resident: fp8 DoubleRow pairs + fp8 16-row K-tail
            # (tail rows replicated at partition bases 0/32/64/96 so the
            # four batch blocks' remainder matmuls run concurrently in
            # distinct PE row-groups; host builds that layout directly).
            # DMA issue order per queue puts the hc=0-gating set (xq, w1
            # piece 0, the K-tails, w2m) ahead of the remaining w1 stream
            # so the main loop starts as soon as the warmup chain ends.
            xq_s = wres.tile([P, KP, 2, BC], FP8, tag="xq")
            xt_s = wres.tile([P, BC], FP8, tag="xt")
            w2m = singles.tile([P, HCP, 2, OUT_PAD], FP8, tag="w2m")
            w1p = [wres.tile([P, KP, 2, P * HC_PER_PIECE], FP8,
                              tag=f"w1_{i}", name=f"w1_{i}")
                   for i in range(W1_PIECES)]
            w1tp = [wres.tile([P, HIDDEN // 4], FP8, tag=f"w1t_{i}",
                              name=f"w1t_{i}")
                    for i in range(4)]

            def w1_cs(i):
                return slice(i * P * HC_PER_PIECE, (i + 1) * P * HC_PER_PIECE)

            def w1t_cs(i):
                return slice(i * HIDDEN // 4, (i + 1) * HIDDEN // 4)

            nc.scalar.dma_start(xq_s[:, 0], xq[:, 0])
            nc.sync.dma_start(w1p[0][:, 0], w1q[:, 0, :, w1_cs(0)])
            nc.gpsimd.dma_start(xq_s[:, 2], xq[:, 2])
            nc.scalar.dma_start(w1p[0][:, 1], w1q[:, 1, :, w1_cs(0)])
            nc.gpsimd.dma_start(w1p[0][:, 2], w1q[:, 2, :, w1_cs(0)])
            nc.scalar.dma_start(xt_s, xt[:])
            nc.gpsimd.dma_start(w1tp[0], w1t[:, w1t_cs(0)])
            nc.sync.dma_start(xq_s[:, 1], xq[:, 1])
            nc.scalar.dma_start(w2m, w2q[:])
            nc.gpsimd.dma_start(w1p[1], w1q[:, :, :, w1_cs(1)])
            nc.scalar.dma_start(w1p[2], w1q[:, :, :, w1_cs(2)])
            nc.sync.dma_start(w1p[3], w1q[:, :, :, w1_cs(3)])
            nc.gpsimd.dma_start(w1tp[1], w1t[:, w1t_cs(1)])
            nc.scalar.dma_start(w1tp[2], w1t[:, w1t_cs(2)])
            nc.sync.dma_start(w1tp[3], w1t[:, w1t_cs(3)])
            nc.gpsimd.dma_start(w1p[4], w1q[:, :, :, w1_cs(4)])
            nc.scalar.dma_start(w1p[5], w1q[:, :, :, w1_cs(5)])
            nc.sync.dma_start(w1p[6], w1q[:, :, :, w1_cs(6)])
            nc.gpsimd.dma_start(w1p[7], w1q[:, :, :, w1_cs(7)])

            # main compute: hc-outer / bb-inner. Per hc: 12 fp8 DoubleRow
            # matmuls (3 K-pairs x 4 batch blocks), the 4 bf16 16-row
            # K-remainder matmuls concurrently in PE row-groups, relu
            # (split ACT/DVE/Pool), then the deferred logits matmuls.
            lgs = [psl.tile([OUT_PAD, BB], F32, tag=f"lg_{b}", name=f"lg_{b}")
                   for b in range(NBB)]
            prev = []  # previous pair's (htp, j, bb): logits matmuls deferred

            def flush_prev():
                # newest relu tick first: the first logits matmul's wait
                # covers the rest, so Tile elides the other waits and the
                # next chunk's PSUM-slot WAR wait
                for p_ht, p_j, p_bb in reversed(prev):
                    nc.tensor.matmul(lgs[p_bb], w2m[:, p_j, :, :], p_ht,
                                     start=(p_j == 0), stop=(p_j == HCP - 1),
                                     perf_mode=DR)

            for j in range(HCP):
                # fp8 pair tile for layer 2: [:, sub, :] <- relu(h of hc=2j+sub)
                htps = [hpool.tile([P, 2, BB], FP8, tag="htp",
                                   name=f"htp_{j}_{b}") for b in range(NBB)]
                for sub in range(2):
                    hc = 2 * j + sub
                    piece = w1p[hc // HC_PER_PIECE]
                    col = slice((hc % HC_PER_PIECE) * P,
                                (hc % HC_PER_PIECE) * P + P)
                    tpiece = w1tp[hc // 16]
                    tcol = slice((hc % 16) * P, (hc % 16) * P + P)
                    phs = [psh.tile([P, BB], F32, tag="ph",
                                    name=f"ph_{hc}_{b}") for b in range(NBB)]
                    # kp-outer: consecutive matmuls share the stationary
                    for kp in range(KP):
                        for bb in range(NBB):
                            nc.tensor.matmul(
                                phs[bb],
                                piece[:, kp, :, col],
                                xq_s[:, kp, :, bb * BB : (bb + 1) * BB],
                                start=(kp == 0),
                                stop=False,
                                perf_mode=DR,
                            )
                    # the four bf16 K-remainder matmuls run concurrently in
                    # PE row-groups 0/32/64/96
                    for bb in range(NBB):
                        base = 32 * bb
                        nc.tensor.matmul(
                            phs[bb],
                            tpiece[base : base + K_LAST, tcol],
                            xt_s[base : base + K_LAST,
                                 bb * BB : (bb + 1) * BB],
                            start=False,
                            stop=True,
                            tile_position=(base, 0) if base == 96 else None,
                        )
                    for bb in range(NBB):
                        dst = htps[bb][:, sub, :]
                        if bb < 2:
                            nc.scalar.activation(
                                out=dst, in_=phs[bb],
                                func=mybir.ActivationFunctionType.Relu,
                                bias=zb)
                        else:
                            # Pool can't read PSUM; DVE takes the other half
                            nc.vector.tensor_scalar_max(dst, phs[bb], 0.0)
                    if sub == 1:
                        flush_prev()
                        prev = [(htps[bb], j, bb) for bb in range(NBB)]
            flush_prev()

            # tail: log_softmax for all 16 [128,10] tiles, phased to avoid
            # ACT table swaps (all Exp together, one Ln over [128,16]);
            # transpose outputs borrow the "ph" PSUM slots (groups are done)
            lg_sbs = []
            for bb in range(NBB):
                lg_sb = tailp.tile([OUT_DIM, BB], F32, tag=f"lg_sb_{bb}",
                                   name=f"lg_sb_{bb}")
                nc.vector.tensor_copy(lg_sb, lgs[bb][:OUT_DIM, :])
                lg_sbs.append(lg_sb)
            NT = NBB * (BB // P)  # 16 tiles of [128, 10]
            # all 16 transposed logit tiles land in ONE PSUM slot, then the
            # whole chain runs as a handful of batched ops. Logits are
            # O(+-6) so exp can't overflow f32 -- no max-shift needed.
            pt_all = psh.tile([P, BB], F32, tag="ph", name="pt_all")
            for i in range(NT):
                bb, bs = divmod(i, BB // P)
                nc.tensor.transpose(pt_all[:, i * OUT_DIM : (i + 1) * OUT_DIM],
                                    lg_sbs[bb][:, bs * P : (bs + 1) * P],
                                    ident[:OUT_DIM, :OUT_DIM])
            z_all = tailp.tile([P, NT, OUT_DIM], F32, tag="z_all")
            e_all = tailp.tile([P, NT, OUT_DIM], F32, tag="e_all")
            s_all = tailp.tile([P, NT], F32, tag="s_all")
            ls_all = tailp.tile([P, NT], F32, tag="ls_all")
            ot_all = tailp.tile([P, NT, OUT_DIM], F32, tag="ot_all")
            nc.vector.tensor_copy(z_all, pt_all[:, : NT * OUT_DIM])
            nc.scalar.activation(out=e_all, in_=z_all,
                                 func=mybir.ActivationFunctionType.Exp,
                                 bias=zb)
            nc.vector.reduce_sum(out=s_all, in_=e_all,
                                 axis=mybir.AxisListType.X)
            nc.scalar.activation(out=ls_all, in_=s_all,
                                 func=mybir.ActivationFunctionType.Ln, bias=zb)
            for i in range(NT):
                nc.vector.tensor_scalar(out=ot_all[:, i, :],
                                        in0=z_all[:, i, :],
                                        scalar1=ls_all[:, i : i + 1],
                                        scalar2=None,
                                        op0=mybir.AluOpType.subtract)
            nc.gpsimd.dma_start(out[:].rearrange("(i p) o -> p i o", p=P),
                                ot_all)

    nc.compile()
    return nc


_NC = None


def _get_nc():
    global _NC
    if _NC is None:
        _NC = _build_nc()
    return _NC


def _exact_mask(scores):
    """GetSubnetEP mask, bit-exact vs the reference.

    Keeps the top (n - j) entries of |scores| under stable-sort
    (value, flat-index) order, j = int((1-k)*n): entries > t always kept,
    entries == t kept only for the last (count_at_or_below - j) flat
    indices (ascending flat index == reference's stable sort order).
    """
    s32 = np.asarray(scores, dtype=np.float32)
    a = np.abs(s32).ravel()
    n = a.size
    j = int((1.0 - SPARSITY) * n)
    t = np.partition(a, j)[j]
    lt = int((a < t).sum())
    ties = np.flatnonzero(a == t)  # ascending flat index == stable order
    mask = a > t
    mask[ties[j - lt :]] = True
    assert int(mask.sum()) == n - j
    return mask.reshape(s32.shape)


def _prepare_inputs(x, w1, scores1, w2, scores2):
    x = np.asarray(x, dtype=np.float32)
    w1m = np.asarray(w1, np.float32) * _exact_mask(scores1)
    w2m = np.asarray(w2, np.float32) * _exact_mask(scores2)

    # layer-1 weights: fp8 DoubleRow pair layout [128, KP, 2, HIDDEN]
    w1mT = np.ascontiguousarray(w1m.T)               # [784, 8192]
    w1q = np.ascontiguousarray(
        w1mT[: KP * 2 * P].reshape(KP, 2, P, HIDDEN).transpose(2, 0, 1, 3)
    ).astype(_FP8)
    # fp8 K-tail, replicated at partition bases 0/32/64/96 (fp8 keeps the
    # whole PE instruction stream in dual-fp8 mode -- no mode switches)
    w1t = np.zeros((P, HIDDEN), dtype=_FP8)
    for jj in range(4):
        w1t[32 * jj : 32 * jj + K_LAST] = w1mT[KP * 2 * P :]
    # layer-2 weights: fp8 DoubleRow pair layout [128, HCP, 2, 10]
    w2qh = np.zeros((P, HCP, 2, OUT_PAD), dtype=_FP8)
    w2qh[:, :, :, :OUT_DIM] = w2m.T.reshape(HCP, 2, P, OUT_DIM).transpose(
        2, 0, 1, 3).astype(_FP8)

    # x: fp8 pairs + bf16 tail, per core batch shard
    xT = np.ascontiguousarray(x.T)                   # [784, 16384]
    xq_full = np.ascontiguousarray(
        xT[: KP * 2 * P].reshape(KP, 2, P, B).transpose(2, 0, 1, 3)
    ).astype(_FP8)
    xt_full = np.zeros((P, B), dtype=_FP8)
    for jj in range(4):
        xt_full[32 * jj : 32 * jj + K_LAST] = xT[KP * 2 * P :]

    common = {"w1q": w1q, "w1t": w1t, "w2q": w2qh}
    in_maps = []
    for c in range(N_CORES):
        m = dict(common)
        m["xq"] = np.ascontiguousarray(xq_full[:, :, :, c * BC : (c + 1) * BC])
        m["xt"] = np.ascontiguousarray(xt_full[:, c * BC : (c + 1) * BC])
        in_maps.append(m)
    return in_maps


def run(inputs, trace=False, **kwargs):
    """Run the kernel; returns (output ndarray, BassKernelResults)."""
    nc = _get_nc()
    in_maps = _prepare_inputs(**inputs)
    res = run_bass_kernel_spmd(nc, in_maps, core_ids=list(range(N_CORES)),
                               trace=trace, **kwargs)
    outp = np.concatenate([r["out"] for r in res.results], axis=0)
    return np.ascontiguousarray(outp.astype(np.float32)), res


def kernel(x, w1, scores1, w2, scores2):
    outp, _ = run(dict(x=x, w1=w1, scores1=scores1, w2=w2, scores2=scores2))
    return outp


# revision 21
# speedup vs baseline: 1.1834x; 1.1834x over previous
# Trainium2 Bass kernel for nn_NetSparse1 (topk_masking).
#
# Computes: log_softmax( relu(x @ (w1*m1).T) @ (w2*m2).T ) where m1/m2 are
# top-50%-|score| masks (GetSubnetEP semantics, stable-sort tie handling).
#
# Strategy (data-parallel over 8 NeuronCores, batch dim sharded):
#   host: compute the exact GetSubnetEP masks (k-th order statistic +
#         stable-sort tie handling) and apply them to the weights, then
#         quantize: layer-1 weights and x to fp8e4 (e4m3) in the PE's
#         DoubleRow pair layout (2x matmul throughput), 16-row K-tail and
#         layer-2 weights in bf16. Masking is a pure function of the
#         (replicated) weights/scores, so no per-batch work happens here.
#   device (per core, 2048 batch rows):
#     main: hc-outer / bb-inner: per 128-hidden chunk and 512-batch block,
#           psum[128h,512b] += w1q_pair.T @ xq_pair via 3 fp8 DoubleRow
#           matmuls (K=256 each, 768 of 784), then the 16-row bf16
#           K-remainder of all 4 batch blocks concurrently in PE row-groups
#           0/32/64/96. relu -> bf16 ht split across ACT/DVE/Pool so no
#           single engine gates the PE. logitsT[10,512] += w2m_chunk.T @ ht,
#           deferred one full chunk so the PE never stalls on the relu.
#           A short bf16 warmup matmul chain keeps the HAM clock-gate at
#           K=8/8 from the start.
#     epilog: batched log_softmax over 16 [128,10] tiles (PE transpose,
#           max-shift, Exp/Ln grouped to avoid ACT table swaps), one DMA.
# No collectives needed; host concatenates the 8 per-core outputs.

import numpy as np
import ml_dtypes

import concourse.bass as bass
import concourse.tile as tile
from concourse import bacc, mybir
from concourse.bass_utils import run_bass_kernel_spmd
from concourse.masks import make_identity

N_CORES = 8
B = 16384
BC = B // N_CORES      # 2048 batch rows per core
IN_DIM = 784
HIDDEN = 8192
OUT_DIM = 10
OUT_PAD = 16          # layer-2 N padded so dual-fp8 ldweights strides stay even
SPARSITY = 0.5

P = 128
KP = 3                 # fp8 DoubleRow K-pairs (3 x 256 = 768 of 784)
K_LAST = IN_DIM - KP * 2 * P  # 16-row bf16 remainder
HC = HIDDEN // P       # 64 hidden chunks
HCP = HC // 2          # 32 hidden chunk pairs (fp8 DoubleRow layer 2)
BB = 512               # batch block (PSUM free dim)
NBB = BC // BB         # 4
W1_PIECES = 8          # w1q DMA pieces along hidden
HC_PER_PIECE = HC // W1_PIECES

F32 = mybir.dt.float32
BF16 = mybir.dt.bfloat16
FP8 = mybir.dt.float8e4

_BF16 = ml_dtypes.bfloat16
_FP8 = ml_dtypes.float8_e4m3

DR = mybir.MatmulPerfMode.DoubleRow


def _build_nc():
    nc = bacc.Bacc("TRN2")

    xq = nc.dram_tensor("xq", (P, KP, 2, BC), FP8, kind="ExternalInput")
    xt = nc.dram_tensor("xt", (P, BC), FP8, kind="ExternalInput")
    w1q = nc.dram_tensor("w1q", (P, KP, 2, HIDDEN), FP8, kind="ExternalInput")
    w1t = nc.dram_tensor("w1t", (P, HIDDEN), FP8, kind="ExternalInput")
    w2q = nc.dram_tensor("w2q", (P, HCP, 2, OUT_PAD), FP8, kind="ExternalInput")
    out = nc.dram_tensor("out", (BC, OUT_DIM), F32, kind="ExternalOutput")

    with tile.TileContext(nc) as tc:
        with (
            tc.tile_pool(name="singles", bufs=1) as singles,
            tc.tile_pool(name="wres", bufs=1) as wres,
            tc.tile_pool(name="hpool", bufs=8) as hpool,
            tc.tile_pool(name="opool", bufs=4) as opool,
            tc.tile_pool(name="tailp", bufs=1) as tailp,
            tc.tile_pool(name="psh", bufs=4, space=bass.MemorySpace.PSUM) as psh,
            tc.tile_pool(name="psl", bufs=1, space=bass.MemorySpace.PSUM) as psl,
        ):
            # zero bias for activations
            zb = singles.tile([P, 1], F32, tag="zb")
            nc.vector.memset(zb, 0.0)

            # identity for PE transpose
            ident = singles.tile([P, P], F32, tag="ident")
            make_identity(nc, ident[:])

            # PE warmup: dependency-free bf16 matmul chain so the HAM
            # clock-gate is at K=8/8 when the first real matmul's inputs land
            wz = singles.tile([P, 2, BB], FP8, tag="wz")
            nc.vector.memset(wz, 0.0)
            warm = psh.tile([P, BB], F32, tag="ph")
            NWARM = 44
            for i in range(NWARM):
                nc.tensor.matmul(warm, wz[:, :, :P], wz, start=(i == 0),
                                 stop=(i == NWARM - 1), perf_mode=DR)

            # x resident: fp8 DoubleRow pairs + fp8 16-row K-tail
            # (tail rows replicated at partition bases 0/32/64/96 so the
            # four batch blocks' remainder matmuls run concurrently in
            # distinct PE row-groups; host builds that layout directly).
            # DMA issue order per queue puts the hc=0-gating set (xq, w1
            # piece 0, the K-tails, w2m) ahead of the remaining w1 stream
            # so the main loop starts as soon as the warmup chain ends.
            xq_s = wres.tile([P, KP, 2, BC], FP8, tag="xq")
            xt_s = wres.tile([P, BC], FP8, tag="xt")
            w2m = singles.tile([P, HCP, 2, OUT_PAD], FP8, tag="w2m")
            w1p = [wres.tile([P, KP, 2, P * HC_PER_PIECE], FP8,
                              tag=f"w1_{i}", name=f"w1_{i}")
                   for i in range(W1_PIECES)]
            w1tp = [wres.tile([P, HIDDEN // 4], FP8, tag=f"w1t_{i}",
                              name=f"w1t_{i}")
                    for i in range(4)]

            def w1_cs(i):
                return slice(i * P * HC_PER_PIECE, (i + 1) * P * HC_PER_PIECE)

            def w1t_cs(i):
                return slice(i * HIDDEN // 4, (i + 1) * HIDDEN // 4)

            nc.scalar.dma_start(xq_s[:, 0], xq[:, 0])
            nc.sync.dma_start(w1p[0], w1q[:, :, :, w1_cs(0)])
            nc.gpsimd.dma_start(xq_s[:, 2], xq[:, 2])
            nc.scalar.dma_start(xt_s, xt[:])
            nc.gpsimd.dma_start(w1tp[0], w1t[:, w1t_cs(0)])
            nc.sync.dma_start(xq_s[:, 1], xq[:, 1])
            nc.scalar.dma_start(w2m, w2q[:])
            nc.gpsimd.dma_start(w1p[1], w1q[:, :, :, w1_cs(1)])
            nc.scalar.dma_start(w1p[2], w1q[:, :, :, w1_cs(2)])
            nc.sync.dma_start(w1p[3], w1q[:, :, :, w1_cs(3)])
            nc.gpsimd.dma_start(w1tp[1], w1t[:, w1t_cs(1)])
            nc.scalar.dma_start(w1tp[2], w1t[:, w1t_cs(2)])
            nc.sync.dma_start(w1tp[3], w1t[:, w1t_cs(3)])
            nc.gpsimd.dma_start(w1p[4], w1q[:, :, :, w1_cs(4)])
            nc.scalar.dma_start(w1p[5], w1q[:, :, :, w1_cs(5)])
            nc.sync.dma_start(w1p[6], w1q[:, :, :, w1_cs(6)])
            nc.gpsimd.dma_start(w1p[7], w1q[:, :, :, w1_cs(7)])

            # main compute: hc-outer / bb-inner. Per hc: 12 fp8 DoubleRow
            # matmuls (3 K-pairs x 4 batch blocks), the 4 bf16 16-row
            # K-remainder matmuls concurrently in PE row-groups, relu
            # (split ACT/DVE/Pool), then the deferred logits matmuls.
            lgs = [psl.tile([OUT_PAD, BB], F32, tag=f"lg_{b}", name=f"lg_{b}")
                   for b in range(NBB)]
            prev = []  # previous pair's (htp, j, bb): logits matmuls deferred

            def flush_prev():
                # newest relu tick first: the first logits matmul's wait
                # covers the rest, so Tile elides the other waits and the
                # next chunk's PSUM-slot WAR wait
                for p_ht, p_j, p_bb in reversed(prev):
                    nc.tensor.matmul(lgs[p_bb], w2m[:, p_j, :, :], p_ht,
                                     start=(p_j == 0), stop=(p_j == HCP - 1),
                                     perf_mode=DR)

            for j in range(HCP):
                # fp8 pair tile for layer 2: [:, sub, :] <- relu(h of hc=2j+sub)
                htps = [hpool.tile([P, 2, BB], FP8, tag="htp",
                                   name=f"htp_{j}_{b}") for b in range(NBB)]
                for sub in range(2):
                    hc = 2 * j + sub
                    piece = w1p[hc // HC_PER_PIECE]
                    col = slice((hc % HC_PER_PIECE) * P,
                                (hc % HC_PER_PIECE) * P + P)
                    tpiece = w1tp[hc // 16]
                    tcol = slice((hc % 16) * P, (hc % 16) * P + P)
                    phs = [psh.tile([P, BB], F32, tag="ph",
                                    name=f"ph_{hc}_{b}") for b in range(NBB)]
                    # kp-outer: consecutive matmuls share the stationary
                    for kp in range(KP):
                        for bb in range(NBB):
                            nc.tensor.matmul(
                                phs[bb],
                                piece[:, kp, :, col],
                                xq_s[:, kp, :, bb * BB : (bb + 1) * BB],
                                start=(kp == 0),
                                stop=False,
                                perf_mode=DR,
                            )
                    # the four bf16 K-remainder matmuls run concurrently in
                    # PE row-groups 0/32/64/96
                    for bb in range(NBB):
                        base = 32 * bb
                        nc.tensor.matmul(
                            phs[bb],
                            tpiece[base : base + K_LAST, tcol],
                            xt_s[base : base + K_LAST,
                                 bb * BB : (bb + 1) * BB],
                            start=False,
                            stop=True,
                            tile_position=(base, 0) if base == 96 else None,
                        )
                    for bb in range(NBB):
                        dst = htps[bb][:, sub, :]
                        if bb < 2:
                            nc.scalar.activation(
                                out=dst, in_=phs[bb],
                                func=mybir.ActivationFunctionType.Relu,
                                bias=zb)
                        else:
                            # Pool can't read PSUM; DVE takes the other half
                            nc.vector.tensor_scalar_max(dst, phs[bb], 0.0)
                    if sub == 1:
                        flush_prev()
                        prev = [(htps[bb], j, bb) for bb in range(NBB)]
            flush_prev()

            # tail: log_softmax for all 16 [128,10] tiles, phased to avoid
            # ACT table swaps (all Exp together, one Ln over [128,16]);
            # transpose outputs borrow the "ph" PSUM slots (groups are done)
            lg_sbs = []
            for bb in range(NBB):
                lg_sb = tailp.tile([OUT_DIM, BB], F32, tag=f"lg_sb_{bb}",
                                   name=f"lg_sb_{bb}")
                nc.vector.tensor_copy(lg_sb, lgs[bb][:OUT_DIM, :])
                lg_sbs.append(lg_sb)
            NT = NBB * (BB // P)  # 16 tiles of [128, 10]
            # all 16 transposed logit tiles land in ONE PSUM slot, then the
            # whole chain runs as a handful of batched ops. Logits are
            # O(+-6) so exp can't overflow f32 -- no max-shift needed.
            pt_all = psh.tile([P, BB], F32, tag="ph", name="pt_all")
            for i in range(NT):
                bb, bs = divmod(i, BB // P)
                nc.tensor.transpose(pt_all[:, i * OUT_DIM : (i + 1) * OUT_DIM],
                                    lg_sbs[bb][:, bs * P : (bs + 1) * P],
                                    ident[:OUT_DIM, :OUT_DIM])
            z_all = tailp.tile([P, NT, OUT_DIM], F32, tag="z_all")
            e_all = tailp.tile([P, NT, OUT_DIM], F32, tag="e_all")
            s_all = tailp.tile([P, NT], F32, tag="s_all")
            ls_all = tailp.tile([P, NT], F32, tag="ls_all")
            ot_all = tailp.tile([P, NT, OUT_DIM], F32, tag="ot_all")
            nc.vector.tensor_copy(z_all, pt_all[:, : NT * OUT_DIM])
            nc.scalar.activation(out=e_all, in_=z_all,
                                 func=mybir.ActivationFunctionType.Exp,
                                 bias=zb)
            nc.vector.reduce_sum(out=s_all, in_=e_all,
                                 axis=mybir.AxisListType.X)
            nc.scalar.activation(out=ls_all, in_=s_all,
                                 func=mybir.ActivationFunctionType.Ln, bias=zb)
            for i in range(NT):
                nc.vector.tensor_scalar(out=ot_all[:, i, :],
                                        in0=z_all[:, i, :],
                                        scalar1=ls_all[:, i : i + 1],
                                        scalar2=None,
                                        op0=mybir.AluOpType.subtract)
            nc.gpsimd.dma_start(out[:].rearrange("(i p) o -> p i o", p=P),
                                ot_all)

    nc.compile()
    return nc


_NC = None


def _get_nc():
    global _NC
    if _NC is None:
        _NC = _build_nc()
    return _NC


def _exact_mask(scores):
    """GetSubnetEP mask, bit-exact vs the reference.

    Keeps the top (n - j) entries of |scores| under stable-sort
    (value, flat-index) order, j = int((1-k)*n): entries > t always kept,
    entries == t kept only for the last (count_at_or_below - j) flat
    indices (ascending flat index == reference's stable sort order).
    """
    s32 = np.asarray(scores, dtype=np.float32)
    a = np.abs(s32).ravel()
    n = a.size
    j = int((1.0 - SPARSITY) * n)
    t = np.partition(a, j)[j]
    lt = int((a < t).sum())
    ties = np.flatnonzero(a == t)  # ascending flat index == stable order
    mask = a > t
    mask[ties[j - lt :]] = True
    assert int(mask.sum()) == n - j
    return mask.reshape(s32.shape)


def _prepare_inputs(x, w1, scores1, w2, scores2):
    x = np.asarray(x, dtype=np.float32)
    w1m = np.asarray(w1, np.float32) * _exact_mask(scores1)
    w2m = np.asarray(w2, np.float32) * _exact_mask(scores2)

    # layer-1 weights: fp8 DoubleRow pair layout [128, KP, 2, HIDDEN]
    w1mT = np.ascontiguousarray(w1m.T)               # [784, 8192]
    w1q = np.ascontiguousarray(
        w1mT[: KP * 2 * P].reshape(KP, 2, P, HIDDEN).transpose(2, 0, 1, 3)
    ).astype(_FP8)
    # fp8 K-tail, replicated at partition bases 0/32/64/96 (fp8 keeps the
    # whole PE instruction stream in dual-fp8 mode -- no mode switches)
    w1t = np.zeros((P, HIDDEN), dtype=_FP8)
    for jj in range(4):
        w1t[32 * jj : 32 * jj + K_LAST] = w1mT[KP * 2 * P :]
    # layer-2 weights: fp8 DoubleRow pair layout [128, HCP, 2, 10]
    w2qh = np.zeros((P, HCP, 2, OUT_PAD), dtype=_FP8)
    w2qh[:, :, :, :OUT_DIM] = w2m.T.reshape(HCP, 2, P, OUT_DIM).transpose(
        2, 0, 1, 3).astype(_FP8)

    # x: fp8 pairs + bf16 tail, per core batch shard
    xT = np.ascontiguousarray(x.T)                   # [784, 16384]
    xq_full = np.ascontiguousarray(
        xT[: KP * 2 * P].reshape(KP, 2, P, B).transpose(2, 0, 1, 3)
    ).astype(_FP8)
    xt_full = np.zeros((P, B), dtype=_FP8)
    for jj in range(4):
        xt_full[32 * jj : 32 * jj + K_LAST] = xT[KP * 2 * P :]

    common = {"w1q": w1q, "w1t": w1t, "w2q": w2qh}
    in_maps = []
    for c in range(N_CORES):
        m = dict(common)
        m["xq"] = np.ascontiguousarray(xq_full[:, :, :, c * BC : (c + 1) * BC])
        m["xt"] = np.ascontiguousarray(xt_full[:, c * BC : (c + 1) * BC])
        in_maps.append(m)
    return in_maps


def run(inputs, trace=False, **kwargs):
    """Run the kernel; returns (output ndarray, BassKernelResults)."""
    nc = _get_nc()
    in_maps = _prepare_inputs(**inputs)
    res = run_bass_kernel_spmd(nc, in_maps, core_ids=list(range(N_CORES)),
                               trace=trace, **kwargs)
    outp = np.concatenate([r["out"] for r in res.results], axis=0)
    return np.ascontiguousarray(outp.astype(np.float32)), res


def kernel(x, w1, scores1, w2, scores2):
    outp, _ = run(dict(x=x, w1=w1, scores1=scores1, w2=w2, scores2=scores2))
    return outp
